# revision 10
# baseline (speedup 1.0000x reference)
"""KANFIS forward on 8 NeuronCores, data-parallel over the batch.

v2 over the previous baseline: ONE AllReduce instead of two. BN2 statistics
are computed analytically from the Gram matrix G = sum_b proj proj^T (plus
S1 = sum_b proj), which rides in the same [128,129] collective as the BN1
stats; Q1 = diag(G). Phase 2 and 3 fuse into a single pass (y -> gelu ->
memberships -> head) with no yT/zzT persistents.

Per core (batch shard 16384 rows, 16 chunk-pairs of 1024 columns):

phase1  x load -> 8 PE transposes -> packed ACT [P ; rbf_0] exp -> bf16
        DVE squared-step chain -> per-half (FD 512) paired bf16 matmuls +
        f32r linear matmul -> ACT evict (proj bias + S1 accum) -> per-half
        gram: 4 PE transposes of the just-written projT chunk, Pool evict
        to bf16, 4 accumulating gram matmuls into a persistent PSUM bank.
AR      single AllReduce [128, 129] = [S1 | G].
phase23 header computes a1,d1 (BN1), then S2 = W^T S1 + B*be and
        Q2 = diag(W^T G W) + 2 be (W^T S1) + B be^2 analytically -> a2,d2.
        Pair loop: two y matmuls pack chunk z-blocks at partition bases 0/64
        (zero-padded lhsT rows kill PSUM garbage), ONE Gelu per pair at
        FD 512 covers both chunks, DVE z^2, u1/u2 matmuls with A16*lscale
        prescaled (and base-64 duplicated) fuzzy weights so the LOWER
        memberships come from a one-op int16 bit-trick exp (add, max-0)
        while the uppers are ACT exps with per-partition rescale; head is
        4 bf16 matmul streams per half into a [1,1024] PSUM row, with
        head_b folded in via a constant row appended to the e2u tile.

All parameters are baked into the NEFF as inline tensors (two blob DMAs);
only x is a runtime input.
"""
import numpy as np
import ml_dtypes
from contextlib import ExitStack

import concourse.bass as bass
import concourse.tile as tile
from concourse import mybir
from concourse.vector_clock import ScopedClock
from concourse.bass_utils import run_bass_kernel_spmd

F32 = mybir.dt.float32
F32R = mybir.dt.float32r
BF16 = mybir.dt.bfloat16
I16 = mybir.dt.int16
AF = mybir.ActivationFunctionType
ALU = mybir.AluOpType
BF = ml_dtypes.bfloat16

NCORES = 8
B = 131072
BS = B // NCORES          # 16384 rows per core
G, GS, K, O = 8, 8, 8, 16
TOT, R, FIN = 128, 10, 20
EPS = 1e-5
FC = 512                  # chunk free size
NCH = BS // FC            # 32 chunks
ND = NCH // 2             # 16 pairs
A16 = 128.0 / np.log(2.0)
B16 = 16256.0 - 486411.0 / 65536.0


class SplitDrainTileContext(tile.TileContext):
    """walrus on this stack rejects >1 sync wait per instruction; split the
    kernel-tail drain's waits into single-wait nops."""

    def _drain_and_barrier(self, tick_clock, wait_clock):
        nc = self.nc
        nop = nc.sync.nop(nofuse=True)
        wait_clock.add_sem_waits(nop.ins, ScopedClock({None: tick_clock.global_clock}))
        si = nop.ins.sync_info
        waits = list(si.on_wait) if si and si.on_wait else []
        if len(waits) > 1:
            nop.ins.sync_info = mybir.SyncInfo(on_wait=waits[:1], on_update=si.on_update)
            for w in waits[1:]:
                n2 = nc.sync.nop(nofuse=True)
                n2.ins.sync_info = mybir.SyncInfo(on_wait=[w], on_update=[])
        nc.sync.drain()
        nc.all_engine_barrier()
        assert self.sems is not None
        popped = nc._tile_sem_poison_stack.pop()
        assert popped is self._sem_poison
        nc.clear_and_free_semaphores(list(self.sems.allocated().values()))
        nc.all_engine_barrier()


class _Blob:
    """Pack many small [rows<=128, cols] constants into one inline tensor
    (one DMA). Slices come back as views of a single SBUF tensor."""

    def __init__(self, np_dtype):
        self.np_dtype = np_dtype
        self.cols = []          # (name, rows, c0, width)
        self.arrs = []
        self.off = 0
        self.sb = None

    def add(self, name, arr):
        a = np.ascontiguousarray(np.asarray(arr, np.float64))
        assert a.ndim == 2 and a.shape[0] <= 128
        pad = np.zeros((128, a.shape[1]), np.float64)
        pad[:a.shape[0]] = a
        self.cols.append((name, a.shape[0], self.off, a.shape[1]))
        self.arrs.append(pad)
        self.off += a.shape[1]
        return name

    def build(self, nc, name, sb):
        blob = np.concatenate(self.arrs, axis=1).astype(self.np_dtype)
        c = nc.inline_tensor(blob, name=name)
        self.sb = sb
        return c, blob.shape

    def view(self, name):
        for n, rows, c0, w in self.cols:
            if n == name:
                return self.sb[0:rows, c0:c0 + w]
        raise KeyError(name)


def _build(p):
    nc = bass.Bass(num_devices=NCORES)
    x = nc.dram_tensor("x", [BS, 64], F32, kind="ExternalInput")
    out = nc.dram_tensor("out", [BS, 1], F32, kind="ExternalOutput")
    ar_in = nc.dram_tensor("ar_in", [128, 129], F32)
    ar_out = nc.dram_tensor("ar_out", [128, 129], F32)

    # ---- baked constants (numpy) ----
    sig = np.exp(np.asarray(p["rbf_log_widths"], np.float64)) + 1e-6   # [G,K]
    cen = np.asarray(p["rbf_centres"], np.float64)                     # [G,K]
    inv = 1.0 / sig
    dcen = np.diff(cen, axis=1)
    rec_ok = (K >= 3 and np.allclose(dcen, dcen[:, :1], rtol=1e-5, atol=1e-7)
              and np.allclose(sig, sig[:, :1], rtol=1e-5, atol=1e-9))
    sqs = np.zeros((4, 128), np.float64)
    sqb = np.zeros((4, 128), np.float64)
    for j in range(K // 2):
        for half, k in ((0, 2 * j), (1, 2 * j + 1)):
            sqs[j, half * 64:(half + 1) * 64] = np.repeat(inv[:, k], GS)
            sqb[j, half * 64:(half + 1) * 64] = np.repeat(-cen[:, k] * inv[:, k], GS)
    sqs_rec = np.zeros(128, np.float64)
    sqb_rec = np.zeros(128, np.float64)
    sqs_rec[:64] = np.repeat(inv[:, 0], GS)
    sqb_rec[:64] = np.repeat(-cen[:, 0] * inv[:, 0], GS)
    dc_g = dcen[:, 0] if rec_ok else np.zeros(G)
    exps_rec = np.zeros(128, np.float64)
    exps_rec[:64] = np.repeat(dc_g / sig[:, 0] ** 2, GS)
    exps_rec[64:] = -0.5
    pw = np.asarray(p["proj_W"], np.float64)            # [G,O,GS]
    w = np.asarray(p["rbf_weights"], np.float64)        # [G,K]
    if rec_ok:
        gam = np.exp(-(cen ** 2 - cen[:, :1] ** 2) / (2 * sig[:, :1] ** 2))
        weff = w * gam
    else:
        weff = w
    lhp = np.zeros((K // 2, 128, 128), np.float64)
    for j in range(K // 2):
        for half, k in ((0, 2 * j), (1, 2 * j + 1)):
            for g in range(G):
                lhp[j, half * 64 + g * GS:half * 64 + (g + 1) * GS,
                    g * O:(g + 1) * O] = pw[g].T * weff[g, k]
    linT = np.zeros((64, 128), np.float64)
    for g in range(G):
        linT[g * GS:(g + 1) * GS, g * O:(g + 1) * O] = (
            pw[g].T * np.asarray(p["rbf_linear_w"], np.float64)[g])
    pbv = np.asarray(p["proj_b"], np.float64).reshape(128, 1)
    # fuzzy layer
    su = np.exp(np.asarray(p["fz_log_su"], np.float64)) + 1e-6          # [R,FIN]
    sl = np.minimum(np.exp(np.asarray(p["fz_log_sl"], np.float64)) + 1e-6, su * 0.9)
    cz = np.asarray(p["fz_centres"], np.float64)
    afz = np.zeros((52, 200), np.float64)
    for r in range(R):
        for f in range(FIN):
            m = r * FIN + f
            afz[f, m] = -2.0 * cz[r, f] / su[r, f] ** 2
            afz[32 + f, m] = 1.0 / su[r, f] ** 2
    ubias = (-0.5 * cz ** 2 / su ** 2).reshape(200)
    lbias = (-0.5 * cz ** 2 / sl ** 2).reshape(200)
    lscale = (-0.5 * (su / sl) ** 2).reshape(200)
    wh = np.repeat(np.asarray(p["head_W"], np.float64).reshape(R, 1) * 0.5 / FIN,
                   FIN, 0)                                              # [200,1]
    head_b = float(np.asarray(p["head_b"]).reshape(-1)[0])

    # u1/u2 matmul weights, columns prescaled by A16*lscale so the lower
    # membership is a one-op int16 bit-trick; duplicated at partition base
    # 64 for the second packed z-chunk.
    afz1p = afz[:, :128] * (A16 * lscale[:128])[None, :]   # [52,128]
    afz2p = afz[:, 128:] * (A16 * lscale[128:])[None, :]   # [52,72]
    afzA = np.zeros((116, 128), np.float64)
    afzA[0:52] = afz1p
    afzA[64:116] = afz1p
    afzB = np.zeros((116, 72), np.float64)
    afzB[0:52] = afz2p
    afzB[64:116] = afz2p
    su1 = (-0.5 / (A16 * lscale[:128])).reshape(128, 1)
    su2 = (-0.5 / (A16 * lscale[128:])).reshape(72, 1)
    ub1 = ubias[:128].reshape(128, 1)
    ub2 = ubias[128:].reshape(72, 1)
    ablb1 = (A16 * lbias[:128] + B16).reshape(128, 1)
    ablb2 = (A16 * lbias[128:] + B16).reshape(72, 1)
    wh2e = np.zeros((97, 1), np.float64)
    wh2e[0:72] = wh[128:]
    wh2e[96, 0] = head_b

    fb = _Blob(np.float32)
    fb.add("crit", np.stack([sqs_rec, sqb_rec, exps_rec, pbv[:, 0]], axis=1))
    fb.add("id", np.eye(128))
    fb.add("sqs", sqs.T)
    fb.add("sqb", sqb.T)
    fb.add("lin", np.concatenate([linT, np.zeros((64, 128))], axis=0))
    fb.add("g1", np.asarray(p["bn1_gamma"]).reshape(128, 1))
    fb.add("b1", np.asarray(p["bn1_beta"]).reshape(128, 1))
    fb.add("g2", np.asarray(p["bn2_gamma"]).reshape(20, 1))
    fb.add("b2", np.asarray(p["bn2_beta"]).reshape(20, 1))
    fb.add("fpw", np.asarray(p["fp_W"]).T)              # [128,20]
    fb.add("fpb", np.asarray(p["fp_b"]).reshape(20, 1))
    fb.add("su1", su1)
    fb.add("ub1", ub1)
    fb.add("ablb1", ablb1)
    fb.add("su2", su2)
    fb.add("ub2", ub2)
    fb.add("ablb2", ablb2)
    fb.add("ones", np.ones((128, 1)))
    fb.add("eps1", np.full((128, 1), EPS))
    fb.add("eps2", np.full((20, 1), EPS))

    bb = _Blob(BF)
    bb.add("lh", lhp.transpose(1, 0, 2).reshape(128, 4 * 128))
    bb.add("idb", np.eye(128))
    bb.add("afzA", afzA)
    bb.add("afzB", afzB)
    bb.add("wh1", wh[:128])
    bb.add("wh2e", wh2e)

    octx = ExitStack()

    def sb(n, s, dt=F32):
        return octx.enter_context(nc.sbuf_tensor(n, s, dt))

    c_fb, fshape = fb.build(nc, "c_fblob", None)
    c_bb, bshape = bb.build(nc, "c_bblob", None)
    k_fblob = sb("k_fblob", list(fshape), F32)
    k_bblob = sb("k_bblob", list(bshape), BF16)
    fb.sb = k_fblob
    bb.sb = k_bblob

    projT = sb("projT", [128, BS], BF16)     # 4MB persistent
    scol1 = sb("scol1", [128, NCH])          # per-half S1 accums
    s1a = sb("s1a", [128, 1])
    gramS = sb("gramS", [128, 128])
    arv = sb("arv", [128, 129])
    a1v = sb("a1v", [128, 1]); d1v = sb("d1v", [128, 1])
    fpw64 = sb("fpw64", [128, 64], BF16)
    fpwF = sb("fpwF", [128, 20])
    biasEffv = sb("biasEffv", [20, 1])
    a2p = sb("a2p", [128, 1]); gbp = sb("gbp", [128, 1])
    Vs = sb("Vs", [128, 20])
    e2u_a = sb("e2u_a", [97, 1024], BF16)
    e2u_b = sb("e2u_b", [97, 1024], BF16)
    gram_ctx = ExitStack()
    gram_ps = gram_ctx.enter_context(nc.psum_tensor("gram_ps", [128, 128], F32))

    k_crit = fb.view("crit"); k_id = fb.view("id")
    k_sqs = fb.view("sqs"); k_sqb = fb.view("sqb")
    k_lin = fb.view("lin")[0:64, :]
    k_g1 = fb.view("g1"); k_b1 = fb.view("b1")
    k_g2 = fb.view("g2"); k_b2 = fb.view("b2")
    k_fpw = fb.view("fpw"); k_fpb = fb.view("fpb")
    k_su1 = fb.view("su1"); k_ub1 = fb.view("ub1"); k_ablb1 = fb.view("ablb1")
    k_su2 = fb.view("su2"); k_ub2 = fb.view("ub2"); k_ablb2 = fb.view("ablb2")
    k_ones = fb.view("ones")
    k_e1 = fb.view("eps1"); k_e2 = fb.view("eps2")
    k_lh = bb.view("lh"); k_idb = bb.view("idb")
    k_afzA = bb.view("afzA"); k_afzB = bb.view("afzB")
    k_wh1 = bb.view("wh1"); k_wh2e = bb.view("wh2e")

    # ================= phase 1 =================
    with ExitStack() as ctx:
        tc = ctx.enter_context(SplitDrainTileContext(nc))
        nc.gpsimd.dma_start(out=k_fblob[:], in_=c_fb[:, :])
        nc.gpsimd.dma_start(out=k_bblob[:], in_=c_bb[:, :])
        pool = ctx.enter_context(tc.tile_pool(name="p1", bufs=3))
        ps_uix = ctx.enter_context(tc.tile_pool(name="psu", bufs=1, space="PSUM"))
        ps_pp = ctx.enter_context(tc.tile_pool(name="psp", bufs=2, space="PSUM"))
        ps_pbt = ctx.enter_context(tc.tile_pool(name="psb", bufs=2, space="PSUM"))
        # PE clock warm-up (garbage results, never read)
        warm = ps_pp.tile([1, FC], F32, tag="pp", name="warm")
        for i in range(4):
            nc.tensor.matmul(warm[0:1, 0:256], k_ones[:, 0:1],
                             k_fblob[:, 0:256], start=True, stop=True)
        FC2 = 2 * FC
        xv = x.rearrange("(d p s) f -> d p (s f)", p=128, s=8)
        ngm = 0
        for d in range(ND):
            xt = pool.tile([128, 512], F32, tag="xt")
            nc.sync.dma_start(out=xt[:], in_=xv[d])
            uix = ps_uix.tile([128, FC2], F32, tag="uix")
            for j in range(8):
                nc.tensor.transpose(uix[0:64, j * 128:(j + 1) * 128],
                                    xt[:, j * 64:(j + 1) * 64], k_id[:])
            xts = pool.tile([64, FC2], F32R, tag="xts")
            if d % 2 == 0:
                nc.scalar.copy(xts[:], uix[0:64, :])
            else:
                nc.vector.tensor_copy(xts[:], uix[0:64, :])
            eA = pool.tile([128, FC2], BF16, tag="eA")
            eB = pool.tile([128, FC2], BF16, tag="eB")
            eC = pool.tile([128, FC2], BF16, tag="eC")
            eD = pool.tile([128, FC2], BF16, tag="eD")
            if rec_ok:
                nc.scalar.activation(uix[64:128, :], uix[0:64, :], AF.Square,
                                     bias=k_crit[0:64, 1:2],
                                     scale=k_crit[0:64, 0:1])
                e0 = pool.tile([128, FC2], BF16, tag="e0")
                nc.scalar.activation(e0[:], uix[:], AF.Exp, bias=0.0,
                                     scale=k_crit[:, 2:3])
                ptU = pool.tile([128, FC2], BF16, tag="ptU")
                nc.vector.tensor_copy(ptU[64:128, :], e0[0:64, :])
                pp2 = pool.tile([128, FC2], BF16, tag="pp2")
                nc.vector.tensor_copy(eA[0:64, :], e0[64:128, :])
                nc.vector.tensor_tensor(eA[64:128, :], e0[64:128, :],
                                        ptU[64:128, :], ALU.mult)
                nc.vector.tensor_tensor(pp2[0:64, :], e0[0:64, :], e0[0:64, :],
                                        ALU.mult)
                nc.vector.tensor_copy(pp2[64:128, :], pp2[0:64, :])
                nc.vector.tensor_tensor(eB[:], eA[:], pp2[:], ALU.mult)
                nc.vector.tensor_tensor(eC[:], eB[:], pp2[:], ALU.mult)
                nc.vector.tensor_tensor(eD[:], eC[:], pp2[:], ALU.mult)
            for h in range(2):
                hs = slice(h * FC, (h + 1) * FC)
                pp = ps_pp.tile([128, FC], F32, tag="pp")
                if rec_ok:
                    for j, et in enumerate([eA, eB, eC, eD]):
                        nc.tensor.matmul(pp[:], k_lh[:, j * 128:(j + 1) * 128],
                                         et[:, hs], start=(j == 0), stop=False)
                else:
                    for j in range(K // 2):
                        uj2 = ps_pbt.tile([128, FC], F32, tag="pbt", name="uj2")
                        nc.scalar.activation(uj2[:], uix[:, hs], AF.Square,
                                             bias=k_sqb[:, j:j + 1],
                                             scale=k_sqs[:, j:j + 1])
                        ej2 = pool.tile([128, FC], BF16, tag="ej2", name="ej2")
                        nc.scalar.activation(ej2[:], uj2[:], AF.Exp,
                                             bias=0.0, scale=-0.5)
                        nc.tensor.matmul(pp[:], k_lh[:, j * 128:(j + 1) * 128],
                                         ej2[:], start=(j == 0), stop=False)
                nc.tensor.matmul(pp[:], k_lin[:].bitcast(F32R), xts[:, hs],
                                 start=False, stop=True)
                cbase = d * FC2 + h * FC
                nc.scalar.activation(projT[:, cbase:cbase + FC], pp[:],
                                     AF.Identity, bias=k_crit[:, 3:4], scale=1.0,
                                     accum_out=scol1[:, 2 * d + h:2 * d + h + 1])
                # gram of this half: transpose the 4 projT chunks, evict to
                # bf16, accumulate G in a persistent PSUM bank
                pbt = ps_pbt.tile([128, FC], BF16, tag="pbt")
                for c in range(4):
                    nc.tensor.transpose(pbt[:, c * 128:(c + 1) * 128],
                                        projT[:, cbase + c * 128:cbase + (c + 1) * 128],
                                        k_idb[:])
                pbs = pool.tile([128, FC], BF16, tag="pbs")
                if h == 0:
                    nc.scalar.copy(pbs[:], pbt[:])
                else:
                    nc.vector.tensor_copy(pbs[:], pbt[:])
                for c in range(4):
                    nc.tensor.matmul(gram_ps[:, :],
                                     pbs[:, c * 128:(c + 1) * 128],
                                     pbs[:, c * 128:(c + 1) * 128],
                                     start=(ngm == 0), stop=(ngm == NCH * 4 - 1))
                    ngm += 1
        nc.vector.reduce_sum(s1a[:, 0:1], scol1[:], axis=mybir.AxisListType.X)
        nc.scalar.copy(gramS[:], gram_ps[:, :])
        with nc.allow_non_contiguous_dma(reason="128 x 4B col, once per build"):
            nc.sync.dma_start(out=ar_in[:, 0:1], in_=s1a[:])
        nc.sync.dma_start(out=ar_in[:, 1:129], in_=gramS[:])

    gram_ctx.close()
    with nc.semaphore("cc1") as cs:
        nc.gpsimd.collective_compute(
            "AllReduce", ALU.add, replica_groups=[list(range(NCORES))],
            ins=[ar_in[:, :].opt()], outs=[ar_out[:, :].opt()]).then_inc(cs, 1)
        nc.gpsimd.wait_ge(cs, 1)
        nc.all_engine_barrier()

    # ================= phase 2+3 fused =================
    with ExitStack() as ctx:
        tc = ctx.enter_context(SplitDrainTileContext(nc))
        pool = ctx.enter_context(tc.tile_pool(name="p2", bufs=3))
        ps_yp = ctx.enter_context(tc.tile_pool(name="pyp", bufs=2, space="PSUM"))
        ps_u = ctx.enter_context(tc.tile_pool(name="pu", bufs=1, space="PSUM"))
        # PE warm-up spanning the post-AR header latency
        warm2 = ps_yp.tile([128, FC], F32, tag="yp", name="warm2")
        for i in range(3):
            nc.tensor.matmul(warm2[0:128, 0:128], k_id[:], k_id[:],
                             start=True, stop=True)
        nc.sync.dma_start(out=arv[:], in_=ar_out[:, :])
        # ---- BN1 from S1/diag(G) ----
        S1 = arv[:, 0:1]
        mu = pool.tile([128, 1], F32)
        nc.scalar.mul(mu[:], S1, 1.0 / B)
        dgt = pool.tile([128, 128], F32, name="dgt")
        nc.vector.tensor_tensor(dgt[:], arv[:, 1:129], k_id[:], ALU.mult)
        q1 = pool.tile([128, 1], F32)
        nc.vector.reduce_sum(q1[:], dgt[:], axis=mybir.AxisListType.X)
        mus = pool.tile([128, 1], F32)
        nc.vector.tensor_mul(mus[:], mu[:], mu[:])
        var = pool.tile([128, 1], F32)
        nc.vector.scalar_tensor_tensor(var[:], q1[:], 1.0 / B, mus[:],
                                       ALU.mult, ALU.subtract)
        lnv = pool.tile([128, 1], F32)
        nc.scalar.activation(lnv[:], var[:], AF.Ln, bias=k_e1[:], scale=1.0)
        rst = pool.tile([128, 1], F32)
        nc.scalar.activation(rst[:], lnv[:], AF.Exp, bias=0.0, scale=-0.5)
        nc.vector.tensor_mul(a1v[:], rst[:], k_g1[:])
        t1 = pool.tile([128, 1], F32)
        nc.vector.tensor_mul(t1[:], mu[:], a1v[:])
        nc.vector.scalar_tensor_tensor(d1v[:], t1[:], -1.0, k_b1[:],
                                       ALU.mult, ALU.add)
        nc.vector.memset(fpw64[:], 0.0)
        nc.vector.tensor_scalar(fpw64[:, 0:20], k_fpw[:], a1v[:], None, ALU.mult)
        nc.vector.tensor_scalar(fpwF[:], k_fpw[:], a1v[:], None, ALU.mult)
        bp = ps_u.tile([20, 1], F32, tag="u2", name="bp")
        nc.tensor.matmul(bp[:], k_fpw[:], d1v[:], start=True, stop=True)
        nc.scalar.activation(biasEffv[:], bp[:], AF.Identity, bias=k_fpb[:])
        # ---- BN2 analytics from G ----
        s2v = ps_u.tile([20, 1], F32, tag="u1", name="s2v")
        nc.tensor.matmul(s2v[:], fpwF[:], S1, start=True, stop=True)
        gw = ps_yp.tile([128, 20], F32, tag="yp", name="gw")
        nc.tensor.matmul(gw[:], arv[:, 1:129], fpwF[:], start=True, stop=True)
        nc.vector.tensor_tensor(Vs[:], gw[:], fpwF[:], ALU.mult)
        q2v = ps_u.tile([20, 1], F32, tag="orow", name="q2v")
        nc.tensor.matmul(q2v[:], Vs[:], k_ones[:], start=True, stop=True)
        tb = pool.tile([20, 1], F32)
        nc.vector.tensor_mul(tb[:], biasEffv[:], s2v[:])
        q2 = pool.tile([20, 1], F32)
        nc.vector.scalar_tensor_tensor(q2[:], tb[:], 2.0, q2v[:],
                                       ALU.mult, ALU.add)
        be2 = pool.tile([20, 1], F32)
        nc.vector.tensor_mul(be2[:], biasEffv[:], biasEffv[:])
        q2b = pool.tile([20, 1], F32)
        nc.vector.scalar_tensor_tensor(q2b[:], be2[:], float(B), q2[:],
                                       ALU.mult, ALU.add)
        s2f = pool.tile([20, 1], F32)
        nc.vector.scalar_tensor_tensor(s2f[:], biasEffv[:], float(B), s2v[:],
                                       ALU.mult, ALU.add)
        mu2 = pool.tile([20, 1], F32)
        nc.scalar.mul(mu2[:], s2f[:], 1.0 / B)
        mus2 = pool.tile([20, 1], F32)
        nc.vector.tensor_mul(mus2[:], mu2[:], mu2[:])
        var2 = pool.tile([20, 1], F32)
        nc.vector.scalar_tensor_tensor(var2[:], q2b[:], 1.0 / B, mus2[:],
                                       ALU.mult, ALU.subtract)
        lnv2 = pool.tile([20, 1], F32)
        nc.scalar.activation(lnv2[:], var2[:], AF.Ln, bias=k_e2[:], scale=1.0)
        rst2 = pool.tile([20, 1], F32)
        nc.scalar.activation(rst2[:], lnv2[:], AF.Exp, bias=0.0, scale=-0.5)
        a2v = pool.tile([20, 1], F32)
        nc.vector.tensor_mul(a2v[:], rst2[:], k_g2[:])
        t2 = pool.tile([20, 1], F32)
        nc.vector.tensor_mul(t2[:], mu2[:], a2v[:])
        d2v = pool.tile([20, 1], F32)
        nc.vector.scalar_tensor_tensor(d2v[:], t2[:], -1.0, k_b2[:],
                                       ALU.mult, ALU.add)
        gvb = pool.tile([20, 1], F32)
        nc.vector.tensor_mul(gvb[:], a2v[:], biasEffv[:])
        gbv = pool.tile([20, 1], F32)
        nc.vector.tensor_tensor(gbv[:], gvb[:], d2v[:], ALU.add)
        nc.vector.memset(a2p[:], 0.0)
        nc.vector.memset(gbp[:], 0.0)
        nc.vector.tensor_copy(a2p[0:20, :], a2v[:])
        nc.vector.tensor_copy(a2p[64:84, :], a2v[:])
        nc.vector.tensor_copy(gbp[0:20, :], gbv[:])
        nc.vector.tensor_copy(gbp[64:84, :], gbv[:])
        nc.vector.memset(e2u_a[64:96, :], 0.0)
        nc.vector.memset(e2u_b[64:96, :], 0.0)
        nc.vector.memset(e2u_a[96:97, :], 1.0)
        nc.vector.memset(e2u_b[96:97, :], 1.0)
        # ---- pair loop ----
        ov = out[:, :].rearrange("(q s) one -> q (s one)", s=FC2)
        for d in range(ND):
            base = d * FC2
            yp2 = ps_yp.tile([128, FC], F32, tag="yp")
            nc.tensor.matmul(yp2[0:64, :], fpw64[:], projT[:, base:base + FC],
                             start=True, stop=True)
            nc.tensor.matmul(yp2[64:128, :], fpw64[:],
                             projT[:, base + FC:base + FC2],
                             start=True, stop=True)
            Z = pool.tile([128, FC], BF16, tag="z")
            nc.scalar.activation(Z[:], yp2[:], AF.Gelu, bias=gbp[:],
                                 scale=a2p[:])
            nc.vector.tensor_tensor(Z[32:52, :], Z[0:20, :], Z[0:20, :],
                                    ALU.mult)
            nc.vector.tensor_tensor(Z[96:116, :], Z[64:84, :], Z[64:84, :],
                                    ALU.mult)
            u1 = ps_u.tile([128, FC2], F32, tag="u1")
            u2 = ps_u.tile([72, FC2], F32, tag="u2")
            nc.tensor.matmul(u1[:, 0:FC], k_afzA[0:52, :], Z[0:52, :],
                             start=True, stop=True)
            nc.tensor.matmul(u1[:, FC:FC2], k_afzA[64:116, :], Z[64:116, :],
                             start=True, stop=True)
            nc.tensor.matmul(u2[:, 0:FC], k_afzB[0:52, :], Z[0:52, :],
                             start=True, stop=True)
            nc.tensor.matmul(u2[:, FC:FC2], k_afzB[64:116, :], Z[64:116, :],
                             start=True, stop=True)
            e1u = pool.tile([128, FC2], BF16, tag="e1u")
            nc.scalar.activation(e1u[:], u1[:], AF.Exp, bias=k_ub1[:],
                                 scale=k_su1[:])
            e1l = pool.tile([128, FC2], I16, tag="e1l")
            nc.vector.tensor_scalar(e1l[:], u1[:], k_ablb1[:], 0.0,
                                    ALU.add, ALU.max)
            e2X = e2u_a if d % 2 == 0 else e2u_b
            nc.scalar.activation(e2X[0:72, :], u2[:], AF.Exp, bias=k_ub2[:],
                                 scale=k_su2[:])
            e2l = pool.tile([72, FC2], I16, tag="e2l")
            nc.vector.tensor_scalar(e2l[:], u2[:], k_ablb2[:], 0.0,
                                    ALU.add, ALU.max)
            orow = ps_u.tile([1, FC2], F32, tag="orow")
            for h in range(2):
                hs = slice(h * FC, (h + 1) * FC)
                ohs = orow[:, hs]
                nc.tensor.matmul(ohs, k_wh1[:], e1u[:, hs],
                                 start=True, stop=False)
                nc.tensor.matmul(ohs, k_wh1[:], e1l[:, hs].bitcast(BF16),
                                 start=False, stop=False)
                nc.tensor.matmul(ohs, k_wh2e[:], e2X[0:97, hs],
                                 start=False, stop=False)
                nc.tensor.matmul(ohs, k_wh2e[0:72, :], e2l[:, hs].bitcast(BF16),
                                 start=False, stop=True)
            osb = pool.tile([1, FC2], F32, tag="osb")
            # batch row within pair = 8p + 4h + j  (orow col = h*512+j*128+p)
            nc.vector.tensor_copy(
                osb[:].rearrange("one (p h j) -> one p h j", h=2, j=4),
                orow[:].rearrange("one (h j p) -> one p h j", h=2, j=4))
            nc.sync.dma_start(out=ov[d:d + 1, :], in_=osb[:])
    octx.close()
    _split_multiwaits(nc)
    return nc


def _split_multiwaits(nc, max_waits=1):
    for bb_ in nc.m.functions[0].blocks:
        insts = bb_.instructions
        i = 0
        while i < len(insts):
            inst = insts[i]
            si = getattr(inst, "sync_info", None)
            waits = list(si.on_wait) if si and si.on_wait else []
            if len(waits) > max_waits:
                inst.sync_info = mybir.SyncInfo(
                    on_wait=waits[:max_waits], on_update=si.on_update)
                for j, w in enumerate(waits[max_waits:]):
                    n = mybir.InstNoOp(name=f"{inst.name}_ws{j}", ins=[], outs=[])
                    n.engine = inst.engine
                    n.sync_info = mybir.SyncInfo(on_wait=[w], on_update=[])
                    nc.register_instruction(n, overwrite=True)
                    insts.insert(i, n)
                    i += 1
            i += 1


LAST_RESULTS = None


def kernel(**inputs):
    global LAST_RESULTS
    import os
    x = np.asarray(inputs["x"], np.float32)
    p = {k: np.asarray(v) for k, v in inputs.items() if k != "x"}
    nc = _build(p)
    in_maps = [{"x": np.ascontiguousarray(x[i * BS:(i + 1) * BS])}
               for i in range(NCORES)]
    kw = {}
    if os.environ.get("KANFIS_TRACE") == "1":
        kw = dict(trace=True, tmpdir=os.environ.get("KANFIS_TRACE_DIR") or None)
    res = run_bass_kernel_spmd(nc, in_maps, core_ids=list(range(NCORES)), **kw)
    LAST_RESULTS = res
    return np.concatenate([res.results[i]["out"] for i in range(NCORES)], axis=0)
